# revision 40
# baseline (speedup 1.0000x reference)
"""LoRA generator kernel for Trainium2, sharded over 8 NeuronCores by layer.

Reference computation (see problem):
  pe = (condition @ W_proj + b_proj)                        (B=2, 224, 512)
  A  = (gelu(pe@WA1+bA1) @ WA2 + bA2) -> (B, L, 7, 16, 64)
  Bm = (gelu(pe@WB1+bB1) @ WB2 + bB2) -> (B, L, 7, 64, 16)
  out per (b, layer): concat over t of [tile_cols(A)*scA (16 x in_d),
                                        tile_rows(B)*scB (out_d x 16)]

Each core handles 4 layers (28 of the 224 projections), bf16 end-to-end
(out is bf16, upcast on host). Per core ~11MB W_proj read + ~18.4MB out
write; the kernel aims to keep the 16 SDMA engines saturated.

Design notes:
  - pe: cond-stationary matmuls into one [16,512] PSUM bank (row pairs per
    t), one DVE add folds b_proj, then 4 XBAR DMA-transposes make peT.
  - Both decoders run mm2 with the per-row h vector replicated 16x in the
    stationary free dim via a stride-0 broadcast AP, so each row's decoder
    output lands replicated on 16 stride-7 partitions (partition 7d+row%7,
    all 16 SBUF ports). A stride-0 scalar_tensor_tensor applies
    scale*bias + psum in one op (no bias constants from HBM).
  - A pieces: one raw combined-stride DMA slices rank-row r's 64-col chunk
    from partition 7r+s (col 64r) into the aexp base, a bridge DMA doubles
    it, one broadcast copy widens to 4096; pieces go out with 8KB
    descriptors, b-pairs merged into single 3-dim DMAs.
  - B pieces: obh holds each row's 4KB block x4 (8KB runs); b-pairs merged.
  - Queues: only sync+scalar (HWDGE) carry drains; gpsimd (SWDGE descgen
    is slow) is used purely as a compute engine for scalar_tensor_tensor /
    muls. Raw-AP DMAs are ordered by per-queue FIFO; tracked anchor reads
    bridge engine-op dependencies into the queues and fence buffer reuse.
"""
import sys

sys.path.insert(0, "/opt/trn_rl_repo")

import numpy as np
import ml_dtypes

import concourse.bass as bass
import concourse.bacc as bacc
import concourse.mybir as mybir
import concourse.tile as tile
from concourse.bass_utils import run_bass_kernel_spmd

F32 = mybir.dt.float32
BF16 = mybir.dt.bfloat16
ACT_FN = mybir.ActivationFunctionType.Gelu  # sim override hook
NPBF16 = ml_dtypes.bfloat16

NCORES = 8
NUM_LAYERS = 32
RANK = 16
PED = 512
EMB = 384
T = 7
L = NUM_LAYERS // NCORES          # 4 layers per core
LT = L * T                        # 28 projections per core
ROWS = 2 * LT                     # 56 rows (b, l, t); row = (l*7+t)*2 + b
WP_COLS = LT * PED                # 14336
RPL = 2 * T                       # 14 rows per layer

IN_DS = [4096, 4096, 4096, 4096, 4096, 4096, 11008]
OUT_DS = [4096, 1024, 1024, 4096, 11008, 11008, 4096]
A_SIZES = [16 * d for d in IN_DS]
B_SIZES = [16 * d for d in OUT_DS]
LAYER_SIZE = sum(A_SIZES) + sum(B_SIZES)   # 1150976
OFF_A = []
OFF_B = []
_o = 0
for _t in range(T):
    OFF_A.append(_o)
    _o += A_SIZES[_t]
    OFF_B.append(_o)
    _o += B_SIZES[_t]
OUT_SZ = 2 * L * LAYER_SIZE

RCOLS = T * PED                   # 3584 W_proj columns per layer

MULT = mybir.AluOpType.mult
ADD = mybir.AluOpType.add

PB_L = [0, 32, 64, 64]            # oa partition base per layer (PE quadrants)
ACOL = [0, 0, 0, 1024]            # oa column block per layer


def _build_nc():
    nc = bacc.Bacc(None, target_bir_lowering=False, debug=False)

    cond = nc.declare_dram_parameter("cond", [128, 6], BF16, isOutput=False)
    wp = nc.declare_dram_parameter("wp", [EMB, WP_COLS], BF16, isOutput=False)
    wa1 = nc.declare_dram_parameter("wa1", [128, 1024], BF16, isOutput=False)
    wb1 = nc.declare_dram_parameter("wb1", [128, 1024], BF16, isOutput=False)
    wa2 = nc.declare_dram_parameter("wa2", [128, 2048], BF16, isOutput=False)
    wb2 = nc.declare_dram_parameter("wb2", [128, 2048], BF16, isOutput=False)
    ba1 = nc.declare_dram_parameter("ba1", [128, 2], F32, isOutput=False)
    bb1 = nc.declare_dram_parameter("bb1", [128, 2], F32, isOutput=False)
    sca = nc.declare_dram_parameter("sca", [128, ROWS], BF16, isOutput=False)
    scb = nc.declare_dram_parameter("scb", [128, ROWS], BF16, isOutput=False)
    bp2r = nc.declare_dram_parameter("bp2r", [2, WP_COLS], BF16, isOutput=False)
    ba2r = nc.declare_dram_parameter("ba2r", [128, 1024], BF16, isOutput=False)
    bb2r = nc.declare_dram_parameter("bb2r", [128, 1024], BF16, isOutput=False)
    scav = nc.declare_dram_parameter("scav", [128, L], BF16, isOutput=False)
    scbv = nc.declare_dram_parameter("scbv", [128, 2 * L], BF16, isOutput=False)
    ident = nc.declare_dram_parameter("ident", [128, 2], F32, isOutput=False)
    out = nc.declare_dram_parameter("out", [OUT_SZ], BF16, isOutput=True)

    with tile.TileContext(nc) as tc:
        with (
            tc.tile_pool(name="const", bufs=1) as cpool,
            tc.tile_pool(name="wp", bufs=4) as wpool,
            tc.tile_pool(name="work", bufs=1) as wkpool,
            tc.tile_pool(name="ps", bufs=1, space="PSUM") as ps,
        ):
            # ---- loads: sync gets cond + wp0 + wp1; scalar gets the small
            # consts then wp2 + wp3 ----
            cond_sb = cpool.tile([128, 6], BF16)
            nc.sync.dma_start(cond_sb[:], cond[:])
            wp_tiles = []
            for rd in range(L):
                wp_t = wpool.tile(
                    [128, 3 * RCOLS], BF16, tag="wp", name=f"wp{rd}"
                )
                pwt = wp_t[:, :].ap[0][0]
                wp_src = bass.AP(
                    wp, rd * RCOLS,
                    [[WP_COLS, 128], [128 * WP_COLS, 3], [1, RCOLS]],
                )
                wp_dst = bass.AP(
                    wp_t[:, :].tensor, 0, [[pwt, 128], [RCOLS, 3], [1, RCOLS]]
                )
                wp_tiles.append((wp_t, wp_dst, wp_src))
            nc.sync.dma_start(wp_tiles[0][1], wp_tiles[0][2])

            wa1_sb = cpool.tile([128, 1024], BF16)
            nc.scalar.dma_start(wa1_sb[:], wa1[:])
            wb1_sb = cpool.tile([128, 1024], BF16)
            nc.scalar.dma_start(wb1_sb[:], wb1[:])
            ba1_sb = cpool.tile([128, 2], F32)
            nc.scalar.dma_start(ba1_sb[:], ba1[:])
            bb1_sb = cpool.tile([128, 2], F32)
            nc.scalar.dma_start(bb1_sb[:], bb1[:])
            sca_sb = cpool.tile([128, ROWS], BF16)
            nc.scalar.dma_start(sca_sb[:], sca[:])
            scb_sb = cpool.tile([128, ROWS], BF16)
            nc.scalar.dma_start(scb_sb[:], scb[:])
            bp2r_sb = cpool.tile([2, WP_COLS], BF16)
            nc.scalar.dma_start(bp2r_sb[:], bp2r[:])
            ba2r_sb = cpool.tile([128, 1024], BF16)
            nc.scalar.dma_start(ba2r_sb[:], ba2r[:])
            bb2r_sb = cpool.tile([128, 1024], BF16)
            nc.scalar.dma_start(bb2r_sb[:], bb2r[:])
            scav_sb = cpool.tile([128, L], BF16)
            nc.scalar.dma_start(scav_sb[:], scav[:])
            scbv_sb = cpool.tile([128, 2 * L], BF16)
            nc.scalar.dma_start(scbv_sb[:], scbv[:])
            wa2_sb = cpool.tile([128, 2048], BF16)
            nc.scalar.dma_start(wa2_sb[:], wa2[:])
            wb2_sb = cpool.tile([128, 2048], BF16)
            nc.scalar.dma_start(wb2_sb[:], wb2[:])
            ident_sb = cpool.tile([128, 2], F32)
            nc.scalar.dma_start(ident_sb[:], ident[:])

            nc.sync.dma_start(wp_tiles[1][1], wp_tiles[1][2])
            nc.scalar.dma_start(wp_tiles[2][1], wp_tiles[2][2])
            nc.scalar.dma_start(wp_tiles[3][1], wp_tiles[3][2])

            # ---- long-lived work tiles ----
            pe_sb = [
                wkpool.tile([128, 16 * L], BF16, tag=f"pe_sb{mc}", name=f"pe_sb{mc}")
                for mc in range(4)
            ]
            # A decoder out: layer l on partitions PB_L[l], col block ACOL[l]
            oa = wkpool.tile([128, 2048], BF16, tag="oa", name="oa")
            aexp_bufs = [
                wkpool.tile([128, 4096], BF16, tag=f"aexpb{i}", name=f"aexpb{i}")
                for i in range(4)
            ]
            obh = [
                [
                    wkpool.tile(
                        [128, 1024], BF16, tag=f"obh{h}{p}", name=f"obh{h}{p}"
                    )
                    for p in range(2)
                ]
                for h in range(2)
            ]
            anchor_sb = wkpool.tile([128, 64], BF16, tag="anchor", name="anchor_sb")

            LLS = L * LAYER_SIZE

            def a_pieces(qe, aexp, l, par):
                aexp_t = aexp[:, :].tensor
                pax = aexp[:, :].ap[0][0]

                def base(t, b):
                    return (b * L + l) * LAYER_SIZE + OFF_A[t]

                for s in range(7):
                    row = 7 * par + s
                    t, b = row // 2, row % 2
                    bs = base(t, b)
                    if IN_DS[t] == 4096:
                        src_ = bass.AP(
                            aexp_t, s * pax, [[7 * pax, 16], [1, 4096]]
                        )
                        dst = bass.AP(out, bs, [[4096, 16], [1, 4096]])
                        qe.dma_start(dst, src_)
                    else:  # 11008 = 2*4096 + 2816
                        src_ = bass.AP(
                            aexp_t, s * pax, [[7 * pax, 16], [0, 2], [1, 4096]]
                        )
                        dst = bass.AP(
                            out, bs, [[11008, 16], [4096, 2], [1, 4096]]
                        )
                        qe.dma_start(dst, src_)
                        src_ = bass.AP(
                            aexp_t, s * pax, [[7 * pax, 16], [1, 2816]]
                        )
                        dst = bass.AP(out, bs + 8192, [[11008, 16], [1, 2816]])
                        qe.dma_start(dst, src_)

            def b_pieces(qb, qb2, tgt, l, half):
                # qb2 (if set) takes the t5 pieces for queue-byte balance
                tgt_t = tgt[:, :].tensor
                pobh = tgt[:, :].ap[0][0]

                def base(t, b):
                    return (b * L + l) * LAYER_SIZE + OFF_B[t]

                for i_ in range(7):
                    row = 7 * half + i_
                    t, b = row // 2, row % 2
                    q = qb2 if (qb2 is not None and t == 5) else qb
                    bs = base(t, b)
                    od = OUT_DS[t]
                    if od == 1024:
                        src_ = bass.AP(
                            tgt_t, i_ * pobh, [[7 * pobh, 16], [1, 1024]]
                        )
                        dst = bass.AP(out, bs, [[1024, 16], [1, 1024]])
                        q.dma_start(dst, src_)
                    elif od == 4096:
                        src_ = bass.AP(
                            tgt_t, i_ * pobh,
                            [[7 * pobh, 16], [0, 4], [1, 1024]],
                        )
                        dst = bass.AP(
                            out, bs, [[4096, 16], [1024, 4], [1, 1024]]
                        )
                        q.dma_start(dst, src_)
                    else:  # 11008*16 = 172 blocks = 16*10 + 12
                        src_ = bass.AP(
                            tgt_t, i_ * pobh,
                            [[7 * pobh, 16], [0, 10], [1, 1024]],
                        )
                        dst = bass.AP(
                            out, bs, [[10240, 16], [1024, 10], [1, 1024]]
                        )
                        q.dma_start(dst, src_)
                        src_ = bass.AP(
                            tgt_t, i_ * pobh, [[7 * pobh, 12], [1, 1024]]
                        )
                        dst = bass.AP(
                            out, bs + 163840, [[1024, 12], [1, 1024]]
                        )
                        q.dma_start(dst, src_)

            def decode_layer(l):
                c0s = RPL * l             # scale column base (14-wide blocks)
                c0p = 16 * l              # pe_sb column base (16-wide blocks)

                # A decoder mm1 + gelu + per-row scale
                ha_sb = []
                for mc in range(2):
                    hp = ps.tile([128, RPL], F32, tag=f"h{mc}", name=f"hpa{mc}")
                    for kc in range(4):
                        nc.tensor.matmul(
                            hp[:],
                            wa1_sb[:, kc * 256 + mc * 128 : kc * 256 + (mc + 1) * 128],
                            pe_sb[kc][:, c0p : c0p + RPL],
                            start=(kc == 0),
                            stop=(kc == 3),
                        )
                    hs = wkpool.tile(
                        [128, RPL], BF16, tag=f"h_sb0{mc}", name=f"hsa{mc}"
                    )
                    nc.scalar.activation(
                        hs[:], hp[:], ACT_FN, bias=ba1_sb[:, mc : mc + 1]
                    )
                    nc.vector.tensor_mul(hs[:], hs[:], sca_sb[:, c0s : c0s + RPL])
                    ha_sb.append(hs)

                # A mm2 (both groups at once): stationary = ha [128,14],
                # out rows at the layer's PE-quadrant partitions; bias+scale
                # fused into the psum->oa copy
                pb = PB_L[l]
                acol = ACOL[l]
                for nh in range(2):
                    op = ps.tile([128, 512], F32, tag=f"o{nh}", name=f"opa{nh}")
                    for kc in range(2):
                        nc.tensor.matmul(
                            op[pb : pb + RPL, :],
                            ha_sb[kc][:],
                            wa2_sb[:, kc * 1024 + nh * 512 : kc * 1024 + (nh + 1) * 512],
                            start=(kc == 0),
                            stop=(kc == 1),
                        )
                    nc.vector.scalar_tensor_tensor(
                        oa[pb : pb + RPL, acol + nh * 512 : acol + (nh + 1) * 512],
                        ba2r_sb[pb : pb + RPL, nh * 512 : (nh + 1) * 512],
                        scav_sb[pb : pb + RPL, l : l + 1],
                        op[pb : pb + RPL, :],
                        MULT,
                        ADD,
                    )
                # B decoder
                hb_sb = []
                for mc in range(2):
                    hp = ps.tile([128, RPL], F32, tag=f"h{mc}", name=f"hpb{mc}")
                    for kc in range(4):
                        nc.tensor.matmul(
                            hp[:],
                            wb1_sb[:, kc * 256 + mc * 128 : kc * 256 + (mc + 1) * 128],
                            pe_sb[kc][:, c0p : c0p + RPL],
                            start=(kc == 0),
                            stop=(kc == 3),
                        )
                    hs = wkpool.tile(
                        [128, RPL], BF16, tag=f"h_sb1{mc}", name=f"hsb{mc}"
                    )
                    nc.scalar.activation(
                        hs[:], hp[:], ACT_FN, bias=bb1_sb[:, mc : mc + 1]
                    )
                    nc.vector.tensor_mul(hs[:], hs[:], scb_sb[:, c0s : c0s + RPL])
                    hb_sb.append(hs)
                for half in range(2):
                    tgt = obh[half][l % 2]
                    hd = []
                    for kc in range(2):
                        hdt = wkpool.tile(
                            [128, 112], BF16, tag=f"hdb{kc}{half}",
                            name=f"hdb{kc}{half}",
                        )
                        nc.scalar.copy(
                            hdt[:, 0:112].rearrange("p (d s) -> p d s", s=7),
                            hb_sb[kc][:, 7 * half : 7 * half + 7]
                            .unsqueeze(1)
                            .broadcast_to([128, 16, 7]),
                        )
                        hd.append(hdt)
                    for nh in range(2):
                        op = ps.tile(
                            [128, 512], F32, tag=f"o{nh}", name=f"opb{nh}"
                        )
                        for kc in range(2):
                            nc.tensor.matmul(
                                op[0:112, :],
                                hd[kc][:, 0:112],
                                wb2_sb[:, kc * 1024 + nh * 512 : kc * 1024 + (nh + 1) * 512],
                                start=(kc == 0),
                                stop=(kc == 1),
                            )
                        eng = nc.vector
                        eng.scalar_tensor_tensor(
                            tgt[0:112, nh * 512 : (nh + 1) * 512],
                            bb2r_sb[0:112, nh * 512 : (nh + 1) * 512],
                            scbv_sb[0:112, 2 * l + half : 2 * l + half + 1],
                            op[0:112, :],
                            MULT,
                            ADD,
                        )
                # expansion per group: scatter the 64-col rank chunks to
                # partition 7r+s, bridge, then one broadcast widen
                for par in range(2):
                    g = 2 * l + par
                    gb = pb + 7 * par
                    qe = nc.sync if par == 0 else nc.gpsimd
                    aexp = aexp_bufs[g % 4]
                    aexp_t = aexp[:, :].tensor
                    pax_a = aexp[:, :].ap[0][0]
                    for s in range(7):
                        dst = bass.AP(
                            aexp_t, s * pax_a, [[7 * pax_a, 16], [1, 64]]
                        )
                        qe.dma_start(
                            dst, oa[gb + s : gb + s + 1, acol : acol + 1024]
                        )
                    # bridge the raw writes into tracked deps: double 64->128
                    qe.dma_start(aexp[0:112, 64:128], aexp[0:112, 0:64])
                    # widen 128->4096 in one broadcast copy
                    nc.vector.tensor_copy(
                        aexp[0:112, 128:4096].rearrange(
                            "p (r c) -> p r c", c=128
                        ),
                        aexp[0:112, 0:128].unsqueeze(1).broadcast_to(
                            [112, 31, 128]
                        ),
                    )
                    # gate pieces behind the widening
                    qe.dma_start(
                        anchor_sb[0:1, 8 + par : 9 + par],
                        aexp[0:1, 4095:4096],
                    )
                    a_pieces(qe, aexp, l, par)

                # B drains: h0 on scalar; h1 split gpsimd (+t5 on scalar)
                for half in range(2):
                    tgt = obh[half][l % 2]
                    if half == 0:
                        qs = [nc.scalar]
                        qb, qb2 = nc.scalar, None
                    else:
                        qs = [nc.gpsimd, nc.scalar]
                        qb, qb2 = nc.gpsimd, nc.scalar
                    # gate pieces behind both stt writes (cols 511:513 span
                    # the two nh blocks), on every queue that carries pieces
                    for qi, q in enumerate(qs):
                        q.dma_start(
                            anchor_sb[0:1, 10 + 2 * half + 16 * qi : 12 + 2 * half + 16 * qi],
                            tgt[0:1, 511:513],
                        )
                    b_pieces(qb, qb2, tgt, l, half)
                    # release: obh safe to rewrite after the piece reads
                    for qi, q in enumerate(qs):
                        q.dma_start(
                            anchor_sb[0:1, 14 + 2 * half + 16 * qi : 16 + 2 * half + 16 * qi],
                            tgt[0:1, 511:513],
                        )

            # ---- main pipeline: one layer per round ----
            pe2t_tiles = [
                wkpool.tile([2, PED], F32, tag=f"p2sb{i}", name=f"p2sb{i}")
                for i in range(2)
            ]
            for rd in range(L):
                wp_t = wp_tiles[rd][0]
                # pe for the layer: cond-stationary matmuls (PSUM partitions
                # 0-1), bias fused into the psum->sbuf add, PE transposes
                # build peT in tr_all, then per-chunk copies to pe_sb
                tr_all = ps.tile([128, 64], F32, tag="tra", name=f"tra{rd}")
                for ltl in range(T):
                    p2 = ps.tile([2, PED], F32, tag=f"p2{ltl % 2}", name="pe2_ps")
                    for kc in range(3):
                        nc.tensor.matmul(
                            p2[:],
                            cond_sb[:, kc * 2 : kc * 2 + 2],
                            wp_t[:, kc * RCOLS + ltl * PED : kc * RCOLS + (ltl + 1) * PED],
                            start=(kc == 0),
                            stop=(kc == 2),
                        )
                    pe2t = pe2t_tiles[ltl % 2]
                    nc.vector.tensor_add(
                        pe2t[:],
                        p2[:],
                        bp2r_sb[
                            0:2,
                            rd * RCOLS + ltl * PED : rd * RCOLS + (ltl + 1) * PED,
                        ],
                    )
                    for mc in range(4):
                        nc.tensor.transpose(
                            tr_all[:, mc * 16 + 2 * ltl : mc * 16 + 2 * ltl + 2],
                            pe2t[:, mc * 128 : (mc + 1) * 128],
                            ident_sb[0:2, 0:2],
                        )
                for mc in range(4):
                    nc.vector.tensor_copy(
                        pe_sb[mc][:, 16 * rd : 16 * rd + RPL],
                        tr_all[:, mc * 16 : mc * 16 + RPL],
                    )
                decode_layer(rd)

    nc.finalize()
    return nc


_NC = None


def _get_nc():
    global _NC
    if _NC is None:
        _NC = _build_nc()
    return _NC


def _marshal(inputs):
    """Build the per-core input maps from full inputs."""
    condition = np.asarray(inputs["condition"], np.float32)
    W_proj = np.asarray(inputs["W_proj"], np.float32)
    b_proj = np.asarray(inputs["b_proj"], np.float32)
    WA1 = np.asarray(inputs["WA1"], np.float32)
    bA1 = np.asarray(inputs["bA1"], np.float32)
    WA2 = np.asarray(inputs["WA2"], np.float32)
    bA2 = np.asarray(inputs["bA2"], np.float32)
    WB1 = np.asarray(inputs["WB1"], np.float32)
    bB1 = np.asarray(inputs["bB1"], np.float32)
    WB2 = np.asarray(inputs["WB2"], np.float32)
    bB2 = np.asarray(inputs["bB2"], np.float32)
    scales = np.asarray(inputs["scales"], np.float32)

    cond_arr = np.zeros((128, 6), np.float32)
    for kc in range(3):
        cond_arr[:, kc * 2 : kc * 2 + 2] = condition[:, kc * 128 : (kc + 1) * 128].T
    cond_arr = cond_arr.astype(NPBF16)
    wa1_arr = np.zeros((128, 1024), np.float32)
    wb1_arr = np.zeros((128, 1024), np.float32)
    for kc in range(4):
        wa1_arr[:, kc * 256 : (kc + 1) * 256] = WA1[kc * 128 : (kc + 1) * 128, :]
        wb1_arr[:, kc * 256 : (kc + 1) * 256] = WB1[kc * 128 : (kc + 1) * 128, :]
    wa2_arr = np.zeros((128, 2048), np.float32)
    wb2_arr = np.zeros((128, 2048), np.float32)
    for kc in range(2):
        wa2_arr[:, kc * 1024 : (kc + 1) * 1024] = WA2[kc * 128 : (kc + 1) * 128, :]
        wb2_arr[:, kc * 1024 : (kc + 1) * 1024] = WB2[kc * 128 : (kc + 1) * 128, :]
    wa1_arr = wa1_arr.astype(NPBF16)
    wb1_arr = wb1_arr.astype(NPBF16)
    wa2_arr = wa2_arr.astype(NPBF16)
    wb2_arr = wb2_arr.astype(NPBF16)
    ba1_arr = np.ascontiguousarray(bA1.reshape(2, 128).T)
    bb1_arr = np.ascontiguousarray(bB1.reshape(2, 128).T)
    ba2r_arr = np.broadcast_to(bA2[None, :], (128, 1024)).astype(NPBF16)
    ident_arr = np.zeros((128, 2), np.float32)
    ident_arr[0, 0] = 1.0
    ident_arr[1, 1] = 1.0
    bb2r_arr = np.broadcast_to(bB2[None, :], (128, 1024)).astype(NPBF16)

    in_maps = []
    for c in range(NCORES):
        lt0 = c * LT
        wp_c = np.ascontiguousarray(
            W_proj[:, lt0 * PED : (lt0 + LT) * PED]
        ).astype(NPBF16)
        bp2r_arr = np.broadcast_to(
            b_proj[lt0 * PED : (lt0 + LT) * PED][None, :], (2, WP_COLS)
        ).astype(np.float32)
        sca_row = np.zeros(ROWS, np.float32)
        scb_row = np.zeros(ROWS, np.float32)
        for row in range(ROWS):
            lt = row // 2
            sca_row[row] = scales[lt0 + lt, 0]
            scb_row[row] = scales[lt0 + lt, 1]
        sca_arr = np.broadcast_to(sca_row[None, :], (128, ROWS)).astype(NPBF16)
        scb_arr = np.broadcast_to(scb_row[None, :], (128, ROWS)).astype(NPBF16)
        # per-partition scale vectors:
        # A (oa layout): partition PB_L[l]+row, column l
        # B (replicated layout): partition 7d + row%7, column 2*l + half
        scav_arr = np.zeros((128, L), np.float32)
        scbv_arr = np.zeros((128, 2 * L), np.float32)
        for l in range(L):
            for row in range(RPL):
                scav_arr[PB_L[l] + row, l] = scales[lt0 + l * 7 + row // 2, 0]
            for half in range(2):
                for p in range(112):
                    row = 7 * half + p % 7
                    scbv_arr[p, 2 * l + half] = scales[lt0 + l * 7 + row // 2, 1]
        in_maps.append(
            {
                "cond": cond_arr,
                "wp": wp_c,
                "wa1": wa1_arr,
                "wb1": wb1_arr,
                "wa2": wa2_arr,
                "wb2": wb2_arr,
                "ba1": ba1_arr,
                "bb1": bb1_arr,
                "sca": sca_arr,
                "scb": scb_arr,
                "bp2r": bp2r_arr.astype(NPBF16),
                "ba2r": ba2r_arr,
                "bb2r": bb2r_arr,
                "scav": scav_arr.astype(NPBF16),
                "scbv": scbv_arr.astype(NPBF16),
                "ident": ident_arr,
            }
        )
    return in_maps


def _ensure_ntff_hook():
    """Register the axon NTFF profile hook if the boot didn't (module was
    missing at boot time)."""
    import types

    ah = sys.modules.get("antenv.axon_hooks")
    if ah is None:
        ah = types.ModuleType("antenv.axon_hooks")
        ah._hook = None

        def _set(h, _m=ah):
            _m._hook = h

        def _get(_m=ah):
            return _m._hook

        ah.set_axon_ntff_profile_hook = _set
        ah.get_axon_ntff_profile_hook = _get
        sys.modules["antenv.axon_hooks"] = ah
        import antenv

        antenv.axon_hooks = ah
    if ah.get_axon_ntff_profile_hook() is None:
        if "/root/.axon_site" not in sys.path:
            sys.path.insert(0, "/root/.axon_site")
        from trn_agent_boot.trn_boot import _ntff_profile_via_ctypes

        hook = _ntff_profile_via_ctypes("/opt/axon/libaxon_pjrt.so")
        if hook is not None:
            ah.set_axon_ntff_profile_hook(hook)


def _run(inputs, trace=False):
    if trace:
        _ensure_ntff_hook()
    nc = _get_nc()
    in_maps = _marshal(inputs)
    res = run_bass_kernel_spmd(nc, in_maps, list(range(NCORES)), trace=trace)
    full = np.empty((2, NUM_LAYERS, LAYER_SIZE), np.float32)
    for c in range(NCORES):
        full[:, c * L : (c + 1) * L, :] = (
            res.results[c]["out"].astype(np.float32).reshape(2, L, LAYER_SIZE)
        )
    return full.reshape(2, -1), res


def kernel(**inputs) -> np.ndarray:
    out, _ = _run(inputs, trace=False)
    return out


# revision 41
# speedup vs baseline: 1.1581x; 1.1581x over previous
"""LoRA generator kernel for Trainium2, sharded over 8 NeuronCores by layer.

Reference computation (see problem):
  pe = (condition @ W_proj + b_proj)                        (B=2, 224, 512)
  A  = (gelu(pe@WA1+bA1) @ WA2 + bA2) -> (B, L, 7, 16, 64)
  Bm = (gelu(pe@WB1+bB1) @ WB2 + bB2) -> (B, L, 7, 64, 16)
  out per (b, layer): concat over t of [tile_cols(A)*scA (16 x in_d),
                                        tile_rows(B)*scB (out_d x 16)]

Each core handles 4 layers (28 of the 224 projections), bf16 end-to-end
(out is bf16, upcast on host). Per core ~11MB W_proj read + ~18.4MB out
write; the kernel aims to keep the 16 SDMA engines saturated.

Design notes:
  - pe: cond-stationary matmuls into one [16,512] PSUM bank (row pairs per
    t), one DVE add folds b_proj, then 4 XBAR DMA-transposes make peT.
  - Both decoders run mm2 with the per-row h vector replicated 16x in the
    stationary free dim via a stride-0 broadcast AP, so each row's decoder
    output lands replicated on 16 stride-7 partitions (partition 7d+row%7,
    all 16 SBUF ports). A stride-0 scalar_tensor_tensor applies
    scale*bias + psum in one op (no bias constants from HBM).
  - A pieces: one raw combined-stride DMA slices rank-row r's 64-col chunk
    from partition 7r+s (col 64r) into the aexp base, a bridge DMA doubles
    it, one broadcast copy widens to 4096; pieces go out with 8KB
    descriptors, b-pairs merged into single 3-dim DMAs.
  - B pieces: obh holds each row's 4KB block x4 (8KB runs); b-pairs merged.
  - Queues: only sync+scalar (HWDGE) carry drains; gpsimd (SWDGE descgen
    is slow) is used purely as a compute engine for scalar_tensor_tensor /
    muls. Raw-AP DMAs are ordered by per-queue FIFO; tracked anchor reads
    bridge engine-op dependencies into the queues and fence buffer reuse.
"""
import sys

sys.path.insert(0, "/opt/trn_rl_repo")

import numpy as np
import ml_dtypes

import concourse.bass as bass
import concourse.bacc as bacc
import concourse.mybir as mybir
import concourse.tile as tile
from concourse.bass_utils import run_bass_kernel_spmd

F32 = mybir.dt.float32
BF16 = mybir.dt.bfloat16
ACT_FN = mybir.ActivationFunctionType.Gelu  # sim override hook
NPBF16 = ml_dtypes.bfloat16

NCORES = 8
NUM_LAYERS = 32
RANK = 16
PED = 512
EMB = 384
T = 7
L = NUM_LAYERS // NCORES          # 4 layers per core
LT = L * T                        # 28 projections per core
ROWS = 2 * LT                     # 56 rows (b, l, t); row = (l*7+t)*2 + b
WP_COLS = LT * PED                # 14336
RPL = 2 * T                       # 14 rows per layer

IN_DS = [4096, 4096, 4096, 4096, 4096, 4096, 11008]
OUT_DS = [4096, 1024, 1024, 4096, 11008, 11008, 4096]
A_SIZES = [16 * d for d in IN_DS]
B_SIZES = [16 * d for d in OUT_DS]
LAYER_SIZE = sum(A_SIZES) + sum(B_SIZES)   # 1150976
OFF_A = []
OFF_B = []
_o = 0
for _t in range(T):
    OFF_A.append(_o)
    _o += A_SIZES[_t]
    OFF_B.append(_o)
    _o += B_SIZES[_t]
OUT_SZ = 2 * L * LAYER_SIZE

RCOLS = T * PED                   # 3584 W_proj columns per layer

MULT = mybir.AluOpType.mult
ADD = mybir.AluOpType.add

PB_L = [0, 32, 64, 64]            # oa partition base per layer (PE quadrants)
ACOL = [0, 0, 0, 1024]            # oa column block per layer


def _build_nc():
    nc = bacc.Bacc(None, target_bir_lowering=False, debug=False)

    cond = nc.declare_dram_parameter("cond", [128, 6], BF16, isOutput=False)
    wp = nc.declare_dram_parameter("wp", [EMB, WP_COLS], BF16, isOutput=False)
    wa1 = nc.declare_dram_parameter("wa1", [128, 1024], BF16, isOutput=False)
    wb1 = nc.declare_dram_parameter("wb1", [128, 1024], BF16, isOutput=False)
    wa2 = nc.declare_dram_parameter("wa2", [128, 2048], BF16, isOutput=False)
    wb2 = nc.declare_dram_parameter("wb2", [128, 2048], BF16, isOutput=False)
    ba1 = nc.declare_dram_parameter("ba1", [128, 2], F32, isOutput=False)
    bb1 = nc.declare_dram_parameter("bb1", [128, 2], F32, isOutput=False)
    sca = nc.declare_dram_parameter("sca", [128, ROWS], BF16, isOutput=False)
    scb = nc.declare_dram_parameter("scb", [128, ROWS], BF16, isOutput=False)
    bp2r = nc.declare_dram_parameter("bp2r", [2, WP_COLS], BF16, isOutput=False)
    ba2r = nc.declare_dram_parameter("ba2r", [128, 1024], BF16, isOutput=False)
    bb2r = nc.declare_dram_parameter("bb2r", [128, 1024], BF16, isOutput=False)
    scav = nc.declare_dram_parameter("scav", [128, L], BF16, isOutput=False)
    scbv = nc.declare_dram_parameter("scbv", [128, 2 * L], BF16, isOutput=False)
    ident = nc.declare_dram_parameter("ident", [128, 2], F32, isOutput=False)
    out = nc.declare_dram_parameter("out", [OUT_SZ], BF16, isOutput=True)

    with tile.TileContext(nc) as tc:
        with (
            tc.tile_pool(name="const", bufs=1) as cpool,
            tc.tile_pool(name="wp", bufs=4) as wpool,
            tc.tile_pool(name="work", bufs=1) as wkpool,
            tc.tile_pool(name="ps", bufs=1, space="PSUM") as ps,
        ):
            # ---- loads: sync gets cond + wp0 + wp1; scalar gets the small
            # consts then wp2 + wp3 ----
            cond_sb = cpool.tile([128, 6], BF16)
            nc.sync.dma_start(cond_sb[:], cond[:])
            wp_tiles = []
            for rd in range(L):
                wp_t = wpool.tile(
                    [128, 3 * RCOLS], BF16, tag="wp", name=f"wp{rd}"
                )
                pwt = wp_t[:, :].ap[0][0]
                wp_src = bass.AP(
                    wp, rd * RCOLS,
                    [[WP_COLS, 128], [128 * WP_COLS, 3], [1, RCOLS]],
                )
                wp_dst = bass.AP(
                    wp_t[:, :].tensor, 0, [[pwt, 128], [RCOLS, 3], [1, RCOLS]]
                )
                wp_tiles.append((wp_t, wp_dst, wp_src))
            nc.sync.dma_start(wp_tiles[0][1], wp_tiles[0][2])

            wa1_sb = cpool.tile([128, 1024], BF16)
            nc.scalar.dma_start(wa1_sb[:], wa1[:])
            wb1_sb = cpool.tile([128, 1024], BF16)
            nc.scalar.dma_start(wb1_sb[:], wb1[:])
            ba1_sb = cpool.tile([128, 2], F32)
            nc.scalar.dma_start(ba1_sb[:], ba1[:])
            bb1_sb = cpool.tile([128, 2], F32)
            nc.scalar.dma_start(bb1_sb[:], bb1[:])
            sca_sb = cpool.tile([128, ROWS], BF16)
            nc.scalar.dma_start(sca_sb[:], sca[:])
            scb_sb = cpool.tile([128, ROWS], BF16)
            nc.scalar.dma_start(scb_sb[:], scb[:])
            bp2r_sb = cpool.tile([2, WP_COLS], BF16)
            nc.scalar.dma_start(bp2r_sb[:], bp2r[:])
            ba2r_sb = cpool.tile([128, 1024], BF16)
            nc.scalar.dma_start(ba2r_sb[:], ba2r[:])
            bb2r_sb = cpool.tile([128, 1024], BF16)
            nc.scalar.dma_start(bb2r_sb[:], bb2r[:])
            scav_sb = cpool.tile([128, L], BF16)
            nc.scalar.dma_start(scav_sb[:], scav[:])
            scbv_sb = cpool.tile([128, 2 * L], BF16)
            nc.scalar.dma_start(scbv_sb[:], scbv[:])
            wa2_sb = cpool.tile([128, 2048], BF16)
            nc.scalar.dma_start(wa2_sb[:], wa2[:])
            wb2_sb = cpool.tile([128, 2048], BF16)
            nc.scalar.dma_start(wb2_sb[:], wb2[:])
            ident_sb = cpool.tile([128, 2], F32)
            nc.scalar.dma_start(ident_sb[:], ident[:])

            nc.sync.dma_start(wp_tiles[1][1], wp_tiles[1][2])
            nc.scalar.dma_start(wp_tiles[2][1], wp_tiles[2][2])
            nc.scalar.dma_start(wp_tiles[3][1], wp_tiles[3][2])

            # ---- long-lived work tiles ----
            pe_sb = [
                wkpool.tile([128, 16 * L], BF16, tag=f"pe_sb{mc}", name=f"pe_sb{mc}")
                for mc in range(4)
            ]
            # A decoder out: layer l on partitions PB_L[l], col block ACOL[l]
            oa = wkpool.tile([128, 2048], BF16, tag="oa", name="oa")
            aexp_bufs = [
                wkpool.tile([128, 4096], BF16, tag=f"aexpb{i}", name=f"aexpb{i}")
                for i in range(4)
            ]
            obh = [
                [
                    wkpool.tile(
                        [128, 1024], BF16, tag=f"obh{h}{p}", name=f"obh{h}{p}"
                    )
                    for p in range(2)
                ]
                for h in range(2)
            ]
            anchor_sb = wkpool.tile([128, 64], BF16, tag="anchor", name="anchor_sb")

            LLS = L * LAYER_SIZE

            def a_pieces(qe, aexp, l, par):
                aexp_t = aexp[:, :].tensor
                pax = aexp[:, :].ap[0][0]

                def base(t, b):
                    return (b * L + l) * LAYER_SIZE + OFF_A[t]

                for s in range(7):
                    row = 7 * par + s
                    t, b = row // 2, row % 2
                    bs = base(t, b)
                    if IN_DS[t] == 4096:
                        src_ = bass.AP(
                            aexp_t, s * pax, [[7 * pax, 16], [1, 4096]]
                        )
                        dst = bass.AP(out, bs, [[4096, 16], [1, 4096]])
                        qe.dma_start(dst, src_)
                    else:  # 11008 = 2*4096 + 2816
                        src_ = bass.AP(
                            aexp_t, s * pax, [[7 * pax, 16], [0, 2], [1, 4096]]
                        )
                        dst = bass.AP(
                            out, bs, [[11008, 16], [4096, 2], [1, 4096]]
                        )
                        qe.dma_start(dst, src_)
                        src_ = bass.AP(
                            aexp_t, s * pax, [[7 * pax, 16], [1, 2816]]
                        )
                        dst = bass.AP(out, bs + 8192, [[11008, 16], [1, 2816]])
                        qe.dma_start(dst, src_)

            def b_pieces(qb, qb2, tgt, l, half):
                # qb2 (if set) takes the t5 pieces for queue-byte balance
                tgt_t = tgt[:, :].tensor
                pobh = tgt[:, :].ap[0][0]

                def base(t, b):
                    return (b * L + l) * LAYER_SIZE + OFF_B[t]

                for i_ in range(7):
                    row = 7 * half + i_
                    t, b = row // 2, row % 2
                    q = qb2 if (qb2 is not None and t == 5) else qb
                    bs = base(t, b)
                    od = OUT_DS[t]
                    if od == 1024:
                        src_ = bass.AP(
                            tgt_t, i_ * pobh, [[7 * pobh, 16], [1, 1024]]
                        )
                        dst = bass.AP(out, bs, [[1024, 16], [1, 1024]])
                        q.dma_start(dst, src_)
                    elif od == 4096:
                        src_ = bass.AP(
                            tgt_t, i_ * pobh,
                            [[7 * pobh, 16], [0, 4], [1, 1024]],
                        )
                        dst = bass.AP(
                            out, bs, [[4096, 16], [1024, 4], [1, 1024]]
                        )
                        q.dma_start(dst, src_)
                    else:  # 11008*16 = 172 blocks = 16*10 + 12
                        src_ = bass.AP(
                            tgt_t, i_ * pobh,
                            [[7 * pobh, 16], [0, 10], [1, 1024]],
                        )
                        dst = bass.AP(
                            out, bs, [[10240, 16], [1024, 10], [1, 1024]]
                        )
                        q.dma_start(dst, src_)
                        src_ = bass.AP(
                            tgt_t, i_ * pobh, [[7 * pobh, 12], [1, 1024]]
                        )
                        dst = bass.AP(
                            out, bs + 163840, [[1024, 12], [1, 1024]]
                        )
                        q.dma_start(dst, src_)

            def decode_layer(l):
                c0s = RPL * l             # scale column base (14-wide blocks)
                c0p = 16 * l              # pe_sb column base (16-wide blocks)

                # A decoder mm1 + gelu + per-row scale
                ha_sb = []
                for mc in range(2):
                    hp = ps.tile([128, RPL], F32, tag=f"h{mc}", name=f"hpa{mc}")
                    for kc in range(4):
                        nc.tensor.matmul(
                            hp[:],
                            wa1_sb[:, kc * 256 + mc * 128 : kc * 256 + (mc + 1) * 128],
                            pe_sb[kc][:, c0p : c0p + RPL],
                            start=(kc == 0),
                            stop=(kc == 3),
                        )
                    hs = wkpool.tile(
                        [128, RPL], BF16, tag=f"h_sb0{mc}", name=f"hsa{mc}"
                    )
                    nc.scalar.activation(
                        hs[:], hp[:], ACT_FN, bias=ba1_sb[:, mc : mc + 1]
                    )
                    nc.vector.tensor_mul(hs[:], hs[:], sca_sb[:, c0s : c0s + RPL])
                    ha_sb.append(hs)

                # A mm2 (both groups at once): stationary = ha [128,14],
                # out rows at the layer's PE-quadrant partitions; bias+scale
                # fused into the psum->oa copy
                pb = PB_L[l]
                acol = ACOL[l]
                for nh in range(2):
                    op = ps.tile([128, 512], F32, tag=f"o{nh}", name=f"opa{nh}")
                    for kc in range(2):
                        nc.tensor.matmul(
                            op[pb : pb + RPL, :],
                            ha_sb[kc][:],
                            wa2_sb[:, kc * 1024 + nh * 512 : kc * 1024 + (nh + 1) * 512],
                            start=(kc == 0),
                            stop=(kc == 1),
                        )
                    nc.vector.scalar_tensor_tensor(
                        oa[pb : pb + RPL, acol + nh * 512 : acol + (nh + 1) * 512],
                        ba2r_sb[pb : pb + RPL, nh * 512 : (nh + 1) * 512],
                        scav_sb[pb : pb + RPL, l : l + 1],
                        op[pb : pb + RPL, :],
                        MULT,
                        ADD,
                    )
                # B decoder
                hb_sb = []
                for mc in range(2):
                    hp = ps.tile([128, RPL], F32, tag=f"h{mc}", name=f"hpb{mc}")
                    for kc in range(4):
                        nc.tensor.matmul(
                            hp[:],
                            wb1_sb[:, kc * 256 + mc * 128 : kc * 256 + (mc + 1) * 128],
                            pe_sb[kc][:, c0p : c0p + RPL],
                            start=(kc == 0),
                            stop=(kc == 3),
                        )
                    hs = wkpool.tile(
                        [128, RPL], BF16, tag=f"h_sb1{mc}", name=f"hsb{mc}"
                    )
                    nc.scalar.activation(
                        hs[:], hp[:], ACT_FN, bias=bb1_sb[:, mc : mc + 1]
                    )
                    nc.vector.tensor_mul(hs[:], hs[:], scb_sb[:, c0s : c0s + RPL])
                    hb_sb.append(hs)
                for half in range(2):
                    tgt = obh[half][l % 2]
                    hd = []
                    for kc in range(2):
                        hdt = wkpool.tile(
                            [128, 112], BF16, tag=f"hdb{kc}{half}",
                            name=f"hdb{kc}{half}",
                        )
                        nc.scalar.copy(
                            hdt[:, 0:112].rearrange("p (d s) -> p d s", s=7),
                            hb_sb[kc][:, 7 * half : 7 * half + 7]
                            .unsqueeze(1)
                            .broadcast_to([128, 16, 7]),
                        )
                        hd.append(hdt)
                    for nh in range(2):
                        op = ps.tile(
                            [128, 512], F32, tag=f"o{nh}", name=f"opb{nh}"
                        )
                        for kc in range(2):
                            nc.tensor.matmul(
                                op[0:112, :],
                                hd[kc][:, 0:112],
                                wb2_sb[:, kc * 1024 + nh * 512 : kc * 1024 + (nh + 1) * 512],
                                start=(kc == 0),
                                stop=(kc == 1),
                            )
                        eng = nc.vector
                        eng.scalar_tensor_tensor(
                            tgt[0:112, nh * 512 : (nh + 1) * 512],
                            bb2r_sb[0:112, nh * 512 : (nh + 1) * 512],
                            scbv_sb[0:112, 2 * l + half : 2 * l + half + 1],
                            op[0:112, :],
                            MULT,
                            ADD,
                        )
                # expansion per group: scatter the 64-col rank chunks to
                # partition 7r+s, bridge, then one broadcast widen
                for par in range(2):
                    g = 2 * l + par
                    gb = pb + 7 * par
                    qe = nc.sync if par == 0 else nc.gpsimd
                    aexp = aexp_bufs[g % 4]
                    aexp_t = aexp[:, :].tensor
                    pax_a = aexp[:, :].ap[0][0]
                    for s in range(7):
                        dst = bass.AP(
                            aexp_t, s * pax_a, [[7 * pax_a, 16], [1, 64]]
                        )
                        qe.dma_start(
                            dst, oa[gb + s : gb + s + 1, acol : acol + 1024]
                        )
                    # bridge the raw writes into tracked deps: double 64->128
                    qe.dma_start(aexp[0:112, 64:128], aexp[0:112, 0:64])
                    # widen 128->4096 in one broadcast copy
                    nc.vector.tensor_copy(
                        aexp[0:112, 128:4096].rearrange(
                            "p (r c) -> p r c", c=128
                        ),
                        aexp[0:112, 0:128].unsqueeze(1).broadcast_to(
                            [112, 31, 128]
                        ),
                    )
                    # gate pieces behind the widening
                    qe.dma_start(
                        anchor_sb[0:1, 8 + par : 9 + par],
                        aexp[0:1, 4095:4096],
                    )
                    a_pieces(qe, aexp, l, par)

                # B drains: h0 on scalar, h1 on sync
                for half in range(2):
                    tgt = obh[half][l % 2]
                    qb = nc.scalar if half == 0 else nc.sync
                    # gate pieces behind both stt writes (cols 511:513 span
                    # the two nh blocks)
                    qb.dma_start(
                        anchor_sb[0:1, 10 + 2 * half : 12 + 2 * half],
                        tgt[0:1, 511:513],
                    )
                    b_pieces(qb, None, tgt, l, half)
                    # release: obh safe to rewrite after the piece reads
                    qb.dma_start(
                        anchor_sb[0:1, 14 + 2 * half : 16 + 2 * half],
                        tgt[0:1, 511:513],
                    )

            # ---- main pipeline: one layer per round ----
            pe2t_tiles = [
                wkpool.tile([2, PED], F32, tag=f"p2sb{i}", name=f"p2sb{i}")
                for i in range(2)
            ]
            for rd in range(L):
                wp_t = wp_tiles[rd][0]
                # pe for the layer: cond-stationary matmuls (PSUM partitions
                # 0-1), bias fused into the psum->sbuf add, PE transposes
                # build peT in tr_all, then per-chunk copies to pe_sb
                tr_all = ps.tile([128, 64], F32, tag="tra", name=f"tra{rd}")
                for ltl in range(T):
                    p2 = ps.tile([2, PED], F32, tag=f"p2{ltl % 2}", name="pe2_ps")
                    for kc in range(3):
                        nc.tensor.matmul(
                            p2[:],
                            cond_sb[:, kc * 2 : kc * 2 + 2],
                            wp_t[:, kc * RCOLS + ltl * PED : kc * RCOLS + (ltl + 1) * PED],
                            start=(kc == 0),
                            stop=(kc == 2),
                        )
                    pe2t = pe2t_tiles[ltl % 2]
                    nc.vector.tensor_add(
                        pe2t[:],
                        p2[:],
                        bp2r_sb[
                            0:2,
                            rd * RCOLS + ltl * PED : rd * RCOLS + (ltl + 1) * PED,
                        ],
                    )
                    for mc in range(4):
                        nc.tensor.transpose(
                            tr_all[:, mc * 16 + 2 * ltl : mc * 16 + 2 * ltl + 2],
                            pe2t[:, mc * 128 : (mc + 1) * 128],
                            ident_sb[0:2, 0:2],
                        )
                for mc in range(4):
                    nc.vector.tensor_copy(
                        pe_sb[mc][:, 16 * rd : 16 * rd + RPL],
                        tr_all[:, mc * 16 : mc * 16 + RPL],
                    )
                decode_layer(rd)

    nc.finalize()
    return nc


_NC = None


def _get_nc():
    global _NC
    if _NC is None:
        _NC = _build_nc()
    return _NC


def _marshal(inputs):
    """Build the per-core input maps from full inputs."""
    condition = np.asarray(inputs["condition"], np.float32)
    W_proj = np.asarray(inputs["W_proj"], np.float32)
    b_proj = np.asarray(inputs["b_proj"], np.float32)
    WA1 = np.asarray(inputs["WA1"], np.float32)
    bA1 = np.asarray(inputs["bA1"], np.float32)
    WA2 = np.asarray(inputs["WA2"], np.float32)
    bA2 = np.asarray(inputs["bA2"], np.float32)
    WB1 = np.asarray(inputs["WB1"], np.float32)
    bB1 = np.asarray(inputs["bB1"], np.float32)
    WB2 = np.asarray(inputs["WB2"], np.float32)
    bB2 = np.asarray(inputs["bB2"], np.float32)
    scales = np.asarray(inputs["scales"], np.float32)

    cond_arr = np.zeros((128, 6), np.float32)
    for kc in range(3):
        cond_arr[:, kc * 2 : kc * 2 + 2] = condition[:, kc * 128 : (kc + 1) * 128].T
    cond_arr = cond_arr.astype(NPBF16)
    wa1_arr = np.zeros((128, 1024), np.float32)
    wb1_arr = np.zeros((128, 1024), np.float32)
    for kc in range(4):
        wa1_arr[:, kc * 256 : (kc + 1) * 256] = WA1[kc * 128 : (kc + 1) * 128, :]
        wb1_arr[:, kc * 256 : (kc + 1) * 256] = WB1[kc * 128 : (kc + 1) * 128, :]
    wa2_arr = np.zeros((128, 2048), np.float32)
    wb2_arr = np.zeros((128, 2048), np.float32)
    for kc in range(2):
        wa2_arr[:, kc * 1024 : (kc + 1) * 1024] = WA2[kc * 128 : (kc + 1) * 128, :]
        wb2_arr[:, kc * 1024 : (kc + 1) * 1024] = WB2[kc * 128 : (kc + 1) * 128, :]
    wa1_arr = wa1_arr.astype(NPBF16)
    wb1_arr = wb1_arr.astype(NPBF16)
    wa2_arr = wa2_arr.astype(NPBF16)
    wb2_arr = wb2_arr.astype(NPBF16)
    ba1_arr = np.ascontiguousarray(bA1.reshape(2, 128).T)
    bb1_arr = np.ascontiguousarray(bB1.reshape(2, 128).T)
    ba2r_arr = np.broadcast_to(bA2[None, :], (128, 1024)).astype(NPBF16)
    ident_arr = np.zeros((128, 2), np.float32)
    ident_arr[0, 0] = 1.0
    ident_arr[1, 1] = 1.0
    bb2r_arr = np.broadcast_to(bB2[None, :], (128, 1024)).astype(NPBF16)

    in_maps = []
    for c in range(NCORES):
        lt0 = c * LT
        wp_c = np.ascontiguousarray(
            W_proj[:, lt0 * PED : (lt0 + LT) * PED]
        ).astype(NPBF16)
        bp2r_arr = np.broadcast_to(
            b_proj[lt0 * PED : (lt0 + LT) * PED][None, :], (2, WP_COLS)
        ).astype(np.float32)
        sca_row = np.zeros(ROWS, np.float32)
        scb_row = np.zeros(ROWS, np.float32)
        for row in range(ROWS):
            lt = row // 2
            sca_row[row] = scales[lt0 + lt, 0]
            scb_row[row] = scales[lt0 + lt, 1]
        sca_arr = np.broadcast_to(sca_row[None, :], (128, ROWS)).astype(NPBF16)
        scb_arr = np.broadcast_to(scb_row[None, :], (128, ROWS)).astype(NPBF16)
        # per-partition scale vectors:
        # A (oa layout): partition PB_L[l]+row, column l
        # B (replicated layout): partition 7d + row%7, column 2*l + half
        scav_arr = np.zeros((128, L), np.float32)
        scbv_arr = np.zeros((128, 2 * L), np.float32)
        for l in range(L):
            for row in range(RPL):
                scav_arr[PB_L[l] + row, l] = scales[lt0 + l * 7 + row // 2, 0]
            for half in range(2):
                for p in range(112):
                    row = 7 * half + p % 7
                    scbv_arr[p, 2 * l + half] = scales[lt0 + l * 7 + row // 2, 1]
        in_maps.append(
            {
                "cond": cond_arr,
                "wp": wp_c,
                "wa1": wa1_arr,
                "wb1": wb1_arr,
                "wa2": wa2_arr,
                "wb2": wb2_arr,
                "ba1": ba1_arr,
                "bb1": bb1_arr,
                "sca": sca_arr,
                "scb": scb_arr,
                "bp2r": bp2r_arr.astype(NPBF16),
                "ba2r": ba2r_arr,
                "bb2r": bb2r_arr,
                "scav": scav_arr.astype(NPBF16),
                "scbv": scbv_arr.astype(NPBF16),
                "ident": ident_arr,
            }
        )
    return in_maps


def _ensure_ntff_hook():
    """Register the axon NTFF profile hook if the boot didn't (module was
    missing at boot time)."""
    import types

    ah = sys.modules.get("antenv.axon_hooks")
    if ah is None:
        ah = types.ModuleType("antenv.axon_hooks")
        ah._hook = None

        def _set(h, _m=ah):
            _m._hook = h

        def _get(_m=ah):
            return _m._hook

        ah.set_axon_ntff_profile_hook = _set
        ah.get_axon_ntff_profile_hook = _get
        sys.modules["antenv.axon_hooks"] = ah
        import antenv

        antenv.axon_hooks = ah
    if ah.get_axon_ntff_profile_hook() is None:
        if "/root/.axon_site" not in sys.path:
            sys.path.insert(0, "/root/.axon_site")
        from trn_agent_boot.trn_boot import _ntff_profile_via_ctypes

        hook = _ntff_profile_via_ctypes("/opt/axon/libaxon_pjrt.so")
        if hook is not None:
            ah.set_axon_ntff_profile_hook(hook)


def _run(inputs, trace=False):
    if trace:
        _ensure_ntff_hook()
    nc = _get_nc()
    in_maps = _marshal(inputs)
    res = run_bass_kernel_spmd(nc, in_maps, list(range(NCORES)), trace=trace)
    full = np.empty((2, NUM_LAYERS, LAYER_SIZE), np.float32)
    for c in range(NCORES):
        full[:, c * L : (c + 1) * L, :] = (
            res.results[c]["out"].astype(np.float32).reshape(2, L, LAYER_SIZE)
        )
    return full.reshape(2, -1), res


def kernel(**inputs) -> np.ndarray:
    out, _ = _run(inputs, trace=False)
    return out
